# revision 34
# baseline (speedup 1.0000x reference)
"""Trainium2 Bass kernel for nn_EnhancedFreqLCBlock.

Self-contained: accepts FULL inputs, returns FULL output.
Sharding: 8 cores = 2 batches x 4 quadrant Mamba blocks (expert parallel).
Each core: mask -> quadrant 2D-DCT -> channel LN -> Mamba (hardware
tensor_tensor_scan recurrence) -> residual -> quadrant IDCT contribution.
Host sums the 4 quadrant contributions per batch.
"""
import numpy as np

B, C, H, W = 2, 96, 128, 128
HQ, WQ = H // 2, W // 2          # 64, 64
L = HQ * WQ                      # 4096
D = 192                          # d_inner
S = 16                           # d_state
RK = 6                           # dt_rank
KCONV = 4
NCHUNK = 8
LC = L // NCHUNK                 # 512
NT = (D * S) // 128              # 24 scan partition-tiles
DA, DB = 128, 64                 # d split 192 = 128 + 64

_BUILT = {}


def _dct_mat(N):
    n = np.arange(N)
    M = np.cos(np.pi * (2 * n[None, :] + 1) * n[:, None] / (2 * N)) * np.sqrt(2.0 / N)
    M[0] *= 1.0 / np.sqrt(2.0)
    return M.astype(np.float32)


def _build_nc():
    import concourse.bacc as bacc
    import concourse.bass as bass
    import concourse.mybir as mybir
    import concourse.tile as tile

    f32 = mybir.dt.float32
    bf16 = mybir.dt.bfloat16
    AF = mybir.ActivationFunctionType
    OP = mybir.AluOpType
    AX = mybir.AxisListType
    ts = bass.ts

    nc = bacc.Bacc()

    # ---------------- DRAM I/O ----------------
    xbh = nc.dram_tensor("xbh", [H, C, W], bf16, kind="ExternalInput")
    d_mhqT = nc.dram_tensor("mhqT", [H, HQ], bf16, kind="ExternalInput")
    d_mwqT = nc.dram_tensor("mwqT", [W, WQ], bf16, kind="ExternalInput")
    d_mhq = nc.dram_tensor("mhq", [HQ, H], bf16, kind="ExternalInput")
    d_mwq = nc.dram_tensor("mwq", [WQ, W], bf16, kind="ExternalInput")
    d_ident = nc.dram_tensor("ident", [128, 128], f32, kind="ExternalInput")
    d_inwtap = nc.dram_tensor("inwtap", [C, KCONV * D], bf16,
                              kind="ExternalInput")
    d_inwz = nc.dram_tensor("inwz", [C, D], bf16, kind="ExternalInput")
    d_biasz = nc.dram_tensor("biasz", [D, 1], f32, kind="ExternalInput")
    d_bconv = nc.dram_tensor("bconv", [D, 1], f32, kind="ExternalInput")
    d_bc3 = nc.dram_tensor("bc3", [D, KCONV - 1], f32, kind="ExternalInput")
    d_xpwT = nc.dram_tensor("xpwT", [D, 80], bf16, kind="ExternalInput")
    d_dtwT = nc.dram_tensor("dtwT", [RK, D], bf16, kind="ExternalInput")
    d_dtb = nc.dram_tensor("dtb", [D, 1], f32, kind="ExternalInput")
    d_acol = nc.dram_tensor("acol", [128, NT], f32, kind="ExternalInput")
    d_dpdA = nc.dram_tensor("dpdA", [DA, DA], bf16, kind="ExternalInput")
    d_dpdB = nc.dram_tensor("dpdB", [DB, DB], bf16, kind="ExternalInput")
    d_outwT = nc.dram_tensor("outwT", [D, C], bf16, kind="ExternalInput")
    d_p0164 = nc.dram_tensor("p0164", [128, 128 * 8], bf16, kind="ExternalInput")
    d_s01 = nc.dram_tensor("s01", [S, 128], bf16, kind="ExternalInput")
    d_r01all = nc.dram_tensor("r01all", [128, 128 * 16], bf16, kind="ExternalInput")
    d_r01ball = nc.dram_tensor("r01ball", [128, 64 * 8], bf16, kind="ExternalInput")
    contrib = nc.dram_tensor("contrib", [H, C, W], bf16, kind="ExternalOutput")
    # per-chunk DRAM scratch for the dX row-replication round-trip
    d_dxs = nc.dram_tensor("dxscratch", [NCHUNK // 2, D, 2 * LC], bf16,
                           kind="Internal")

    with tile.TileContext(nc) as tc:
        consts = tc.alloc_tile_pool(name="consts", bufs=1)
        # issue the big input loads before the ~30 const loads: SP
        # dispatches DMAs in program order and the mask path gates startup
        pD_ = tc.alloc_tile_pool(name="pD", bufs=1)
        pB = tc.alloc_tile_pool(name="pB", bufs=1)
        pXH = tc.alloc_tile_pool(name="pXH", bufs=1)
        pA = tc.alloc_tile_pool(name="pA", bufs=1)
        # center row of x (h = H/2): one contiguous descriptor, issued first
        crow = pA.tile([1, C * W], bf16)
        nc.sync.dma_start(crow[:], xbh[H // 2:H // 2 + 1, :, :])
        xh = pXH.tile([H, C * W], bf16)
        xh3 = xh.rearrange("h (c w) -> h c w", c=C)
        # c-chunked loads keep full 128-partition spans (4x the DMA rate of
        # h-chunked loads); x is loaded only once (no c-major copy)
        for i in range(4):
            nc.sync.dma_start(xh3[:, ts(i, 24), :], xbh[:, ts(i, 24), :])

        def cload(dram, shape, dt=f32):
            t = consts.tile(shape, dt, name=f"c_{dram.name}")
            nc.sync.dma_start(t[:], dram[:])
            return t

        def cload2(dram, dt=f32):
            ta = consts.tile([DA] + list(dram.shape[1:]), dt, name=f"cA_{dram.name}")
            nc.sync.dma_start(ta[:], dram[0:DA])
            tb = consts.tile([DB] + list(dram.shape[1:]), dt, name=f"cB_{dram.name}")
            nc.sync.dma_start(tb[:], dram[DA:D])
            return ta, tb

        mhqT = cload(d_mhqT, [H, HQ], bf16)
        mwqT = cload(d_mwqT, [W, WQ], bf16)
        mhq = cload(d_mhq, [HQ, H], bf16)
        mwq64 = consts.tile([128, W], bf16, name="c_mwq64")
        nc.sync.dma_start(mwq64[64:128, :], d_mwq[:])
        ident = cload(d_ident, [128, 128])
        identb = consts.tile([C, C], bf16, name="identb")
        nc.vector.tensor_copy(identb[:], ident[0:C, 0:C])
        inwtap = cload(d_inwtap, [C, KCONV * D], bf16)
        inwz = cload(d_inwz, [C, D], bf16)
        biaszA, biaszB = cload2(d_biasz)
        bconvA, bconvB = cload2(d_bconv)
        bc3A, bc3B = cload2(d_bc3)
        xpwTA, xpwTB = cload2(d_xpwT, bf16)
        dtwT = cload(d_dtwT, [RK, D], bf16)
        dtbA, dtbB = cload2(d_dtb)
        acol = cload(d_acol, [128, NT])
        dpdA = cload(d_dpdA, [DA, DA], bf16)
        dpdB = cload(d_dpdB, [DB, DB], bf16)
        outwTA, outwTB = cload2(d_outwT, bf16)
        p0164 = cload(d_p0164, [128, 128 * 8], bf16)
        s01 = cload(d_s01, [S, 128], bf16)
        r01all = cload(d_r01all, [128, 128 * 16], bf16)
        r01ball = cload(d_r01ball, [128, 64 * 8], bf16)
        onesr = consts.tile([1, 128], f32)
        nc.vector.memset(onesr[:], 1.0)
        onesrb = consts.tile([1, 128], bf16)
        nc.vector.memset(onesrb[:], 1.0)
        ones96b = consts.tile([C, 1], bf16)
        nc.vector.memset(ones96b[:], 1.0)
        eps64 = consts.tile([WQ, 1], f32)
        nc.vector.memset(eps64[:], 1e-5)

        # persistent psum pools (8 banks total: 4 + 2 + 2)
        pmm = tc.alloc_tile_pool(name="pmm", bufs=4, space="PSUM")
        ppy = tc.alloc_tile_pool(name="ppy", bufs=1, space="PSUM")
        ptp = tc.alloc_tile_pool(name="ptp", bufs=1, space="PSUM")

        def mmtile(p, n, nm):
            return pmm.tile([p, n], f32, name=nm, tag="mm")

        def tptile(p, n, nm, dt=f32):
            return ptp.tile([p, n], dt, name=nm, tag="tp")

        # =============== Phase A: mask ===============
        # center pixel vector from the crow strip (strided view, stride W)
        crow3 = crow.rearrange("p (c w) -> p c w", c=C)
        center = crow3[0:1, :, W // 2]                      # [1, C]
        csq = pA.tile([1, C], f32)
        nc.vector.tensor_tensor(csq[:], center, center, op=OP.mult)
        cn = pA.tile([1, 1], f32)
        nc.vector.tensor_reduce(cn[:], csq[:], axis=AX.X, op=OP.add)
        s049 = pA.tile([1, 1], f32)
        nc.vector.tensor_scalar_mul(s049[:], cn[:], 0.49)
        s049p = tptile(128, 1, "s049p")
        nc.tensor.matmul(s049p[:], onesr[:], s049[:], start=True, stop=True)
        s049b = pA.tile([128, 1], f32)
        nc.vector.tensor_copy(s049b[:], s049p[:])
        # center replicated to all 128 partitions
        cenp = tptile(128, C, "cenp")
        nc.tensor.matmul(cenp[:], onesrb[:], center, start=True, stop=True)
        cenb = pA.tile([128, C], bf16)
        nc.vector.tensor_copy(cenb[:], cenp[:])

        num_hw = pA.tile([128, 128], f32)
        ssq_hw = pA.tile([128, 128], f32)
        # per c-chunk (matching the 4 xh3 loads): num = sum_c x*center via an
        # inner-broadcast multiply + middle-axis reduce; ssq = sum_c x^2
        pSq = tc.alloc_tile_pool(name="pSq", bufs=2)
        for g in range(4):
            csl = bass.ds(24 * g, 24)
            prod = pSq.tile([128, 24 * 128], bf16, name="prod")
            prod3 = prod.rearrange("h (c w) -> h c w", c=24)
            nc.vector.tensor_tensor(
                prod3[:, :, :], xh3[:, csl, :],
                cenb[:, csl, None].broadcast_to([128, 24, 128]), op=OP.mult)
            pn = pSq.tile([128, 128], f32, name="pn")
            nc.vector.tensor_reduce(
                pn[:], prod3.transpose([0, 2, 1]), axis=AX.X, op=OP.add)
            sq = pSq.tile([128, 24 * 128], bf16, name="sq")
            sq3 = sq.rearrange("h (c w) -> h c w", c=24)
            nc.scalar.activation(sq3[:, :, :], xh3[:, csl, :], AF.Square)
            psq_ = pSq.tile([128, 128], f32, name="psq_")
            nc.vector.tensor_reduce(
                psq_[:], sq3.transpose([0, 2, 1]), axis=AX.X, op=OP.add)
            if g == 0:
                nc.vector.tensor_copy(num_hw[:], pn[:])
                nc.vector.tensor_copy(ssq_hw[:], psq_[:])
            else:
                nc.vector.tensor_tensor(num_hw[:], num_hw[:], pn[:], op=OP.add)
                nc.vector.tensor_tensor(ssq_hw[:], ssq_hw[:], psq_[:],
                                        op=OP.add)
        pSq.release()

        thr = pA.tile([128, 128], f32)
        nc.scalar.activation(thr[:], ssq_hw[:], AF.Sqrt, bias=0.0, scale=s049b[:])
        nc.vector.tensor_scalar_add(thr[:], thr[:], 0.7e-6)
        mask_hw = pA.tile([128, 128], bf16)
        nc.vector.tensor_tensor(mask_hw[:], num_hw[:], thr[:], op=OP.is_ge)
        for i in range(4):
            nc.vector.tensor_tensor(
                 xh3[:, ts(i, 24), :], xh3[:, ts(i, 24), :],
                 mask_hw[:, None, :].broadcast_to([128, 24, 128]), op=OP.mult)
        pA.release()

        # =============== Phase B: forward DCT ===============
        # t2[w, c, hq] = sum_h x[h, c, w] * Mh_q[hq, h]  (per-c matmul, no
        # separate transpose pass)
        t2 = pB.tile([W, C * HQ], bf16)
        t2_3 = t2.rearrange("p (c q) -> p c q", c=C)
        for c0 in range(0, C, 8):
            tps = tptile(W, 8 * HQ, "tps")
            tps3 = tps.rearrange("p (c q) -> p c q", c=8)
            for k in range(8):
                nc.tensor.matmul(tps3[:, k, :], xh3[:, c0 + k, :], mhqT[:],
                                 start=True, stop=True)
            nc.scalar.activation(t2_3[:, c0:c0 + 8, :], tps3[:, :, :], AF.Copy)
        pXH.release()

        # xdqZ: rows 0:64 = xdq (base 0 for DVE pairing), rows 64:128 = Z
        xdqZ = pD_.tile([128, C * HQ], bf16)
        xdq3 = xdqZ.rearrange("p (c q) -> p c q", c=C)[0:HQ, :, :]
        Z3 = xdqZ.rearrange("p (c q) -> p c q", c=C)[HQ:128, :, :]
        xdq2 = xdqZ[0:HQ, :]
        Z2 = xdqZ[HQ:128, :]
        for i in range(12):
            xps = mmtile(WQ, LC, "xps")
            nc.tensor.matmul(xps[:], mwqT[:], t2[:, ts(i, LC)], start=True, stop=True)
            nc.any.tensor_copy(xdq2[:, ts(i, LC)], xps[:])
        pB.release()

        # =============== Phase C: LayerNorm over c ===============
        pG = tc.alloc_tile_pool(name="pG", bufs=1)
        pE = tc.alloc_tile_pool(name="pE", bufs=1)
        pC = tc.alloc_tile_pool(name="pC", bufs=1)
        # LN stats chunked to the 12 stage-2 copy chunks (8 c's each) so the
        # reduces overlap the DCT matmuls instead of serializing the machine
        smu = pC.tile([WQ, HQ], f32)
        ssq2 = pC.tile([WQ, HQ], f32)
        xn = pC.tile([WQ, C * HQ], bf16)  # first used as xdq^2 scratch
        xn3s = xn.rearrange("p (c q) -> p c q", c=C)
        pPart = tc.alloc_tile_pool(name="pPart", bufs=3)
        for cb in range(C // 8):
            csl = bass.ds(cb * 8, 8)
            nc.vector.tensor_tensor(xn3s[:, csl, :], xdq3[:, csl, :],
                                    xdq3[:, csl, :], op=OP.mult)
            pm = pPart.tile([WQ, HQ], f32, name="pm")
            nc.vector.tensor_reduce(
                pm[:], xdq3[:, csl, :].transpose([0, 2, 1]), axis=AX.X,
                op=OP.add)
            psq = pPart.tile([WQ, HQ], f32, name="psq")
            nc.vector.tensor_reduce(
                psq[:], xn3s[:, csl, :].transpose([0, 2, 1]), axis=AX.X,
                op=OP.add)
            if cb == 0:
                nc.vector.tensor_copy(smu[:], pm[:])
                nc.vector.tensor_copy(ssq2[:], psq[:])
            else:
                nc.vector.tensor_tensor(smu[:], smu[:], pm[:], op=OP.add)
                nc.vector.tensor_tensor(ssq2[:], ssq2[:], psq[:], op=OP.add)
        pPart.release()
        mu = pC.tile([WQ, HQ], f32)
        nc.vector.tensor_scalar_mul(mu[:], smu[:], 1.0 / C)
        var = pC.tile([WQ, HQ], f32)
        nc.vector.tensor_scalar_mul(ssq2[:], ssq2[:], 1.0 / C)
        nc.vector.tensor_tensor(var[:], mu[:], mu[:], op=OP.mult)
        nc.vector.tensor_tensor(var[:], ssq2[:], var[:], op=OP.subtract)
        sd = pC.tile([WQ, HQ], f32)
        nc.scalar.activation(sd[:], var[:], AF.Sqrt, bias=eps64[:])
        inv = pC.tile([WQ, HQ], f32)
        nc.vector.reciprocal(inv[:], sd[:])
        # bf16 stats so the normalize runs at DVE 2x rate; chunk by hq so
        # the transposes start before the whole tensor is normalized
        mub = pC.tile([WQ, HQ], bf16)
        nc.vector.tensor_copy(mub[:], mu[:])
        invb = pC.tile([WQ, HQ], bf16)
        nc.vector.tensor_copy(invb[:], inv[:])
        xn3 = xn.rearrange("p (c q) -> p c q", c=C)
        xn_c = pE.tile([C, L], bf16)
        xi2A = pG.tile([DA, L], bf16)
        zsA = pG.tile([DA, L], bf16)
        xi2B_t = pG.tile([DB, L], bf16, name="xi2B_t")
        zsB_t = pG.tile([DB, L], bf16, name="zsB_t")
        xi2B = xi2B_t[:, :]
        zsB = zsB_t[:, :]

        # ====== fused loop: LN-normalize -> in_proj(+conv taps) -> scan ======
        # (per 1024-col chunk so PE/Scalar front-end work overlaps the
        # DVE-bound scan of the previous chunk)
        pT = tc.alloc_tile_pool(name="pT", bufs=3)
        hlast = pG.tile([128, NT], bf16)
        LCF = 2 * LC
        NI = NCHUNK // 2
        ctxs = [dict() for _ in range(NI)]

        def make_front(i):
            """Front-end of chunk i as a closure list, so it can be emitted
            interleaved into chunk i-1's scan loop."""
            cx = ctxs[i]
            h0 = 16 * i

            def f_norm():
                hsl = bass.ds(h0, 16)
                nc.vector.tensor_tensor(
                    xn3[:, :, hsl], xdq3[:, :, hsl],
                    mub[:, None, hsl].broadcast_to([WQ, C, 16]),
                    op=OP.subtract)
                nc.vector.tensor_tensor(
                    xn3[:, :, hsl], xn3[:, :, hsl],
                    invb[:, None, hsl].broadcast_to([WQ, C, 16]), op=OP.mult)

            def f_trans():
                tps2 = tptile(C, 16 * WQ, "tps2", bf16)
                tps2_3 = tps2.rearrange("p (h q) -> p h q", h=16)
                for k in range(16):
                    nc.tensor.matmul(tps2_3[:, k, :], xn3[:, :, h0 + k],
                                     identb[0:WQ, 0:WQ],
                                     is_transpose=True, start=True, stop=True)
                nc.scalar.activation(xn_c[:, h0 * WQ:(h0 + 16) * WQ], tps2[:],
                                     AF.Copy)

            def f_ip(c2):
                def run():
                    c8 = 2 * i + c2
                    t0 = c8 * LC
                    psA = mmtile(128, LC, "psA")
                    psBz = mmtile(128, LC, "psBz")
                    psZ = mmtile(128, LC, "psZ")
                    for k in range(KCONV):
                        if c8 == 0:
                            rhs = xn_c[:, 0:LC - k]
                            oA = psA[:, k:LC]
                            oB = psBz[0:64, k:LC]
                        else:
                            rhs = xn_c[:, t0 - k:t0 - k + LC]
                            oA = psA[:, :]
                            oB = psBz[0:64, :]
                        nc.tensor.matmul(oA, inwtap[:, k * D:k * D + DA], rhs,
                                         start=(k == 0), stop=(k == KCONV - 1))
                        nc.tensor.matmul(oB, inwtap[:, k * D + DA:(k + 1) * D],
                                         rhs, start=(k == 0),
                                         stop=(k == KCONV - 1),
                                         skip_group_check=True)
                    nc.tensor.matmul(psZ[:], inwz[:, 0:DA], xn_c[:, ts(c8, LC)],
                                     start=True, stop=True)
                    nc.tensor.matmul(psBz[64:128, :], inwz[:, DA:D],
                                     xn_c[:, ts(c8, LC)],
                                     start=True, stop=True,
                                     skip_group_check=True)
                    nc.scalar.activation(xi2A[:, ts(c8, LC)], psA[:], AF.Silu,
                                         bias=bconvA[:])
                    nc.scalar.activation(xi2B[:, ts(c8, LC)], psBz[0:64, :],
                                         AF.Silu, bias=bconvB[:])
                    nc.scalar.activation(zsA[:, ts(c8, LC)], psZ[:], AF.Silu,
                                         bias=biaszA[:])
                    nc.scalar.activation(zsB[:, ts(c8, LC)], psBz[64:128, :],
                                         AF.Silu, bias=biaszB[:])
                    if c8 == 0:
                        # first KCONV-1 cols: truncated tap sum -> redo silu
                        # with the truncated bias
                        for t in range(KCONV - 1):
                            nc.scalar.activation(
                                xi2A[:, t:t + 1], psA[:, t:t + 1], AF.Silu,
                                bias=bc3A[:, t:t + 1])
                            nc.scalar.activation(
                                xi2B[:, t:t + 1], psBz[0:64, t:t + 1], AF.Silu,
                                bias=bc3B[:, t:t + 1])
                return run

            def f_xp(h):
                def run():
                    if h == 0:
                        cx["dt_c"] = pT.tile([RK, LCF], bf16, name="dt_c",
                                             bufs=1)
                        cx["bm_c"] = pT.tile([S, LCF], bf16, name="bm_c",
                                             bufs=1)
                        cx["cm_c"] = pT.tile([S, LCF], bf16, name="cm_c",
                                             bufs=1)
                    dt_c, bm_c, cm_c = cx["dt_c"], cx["bm_c"], cx["cm_c"]
                    sl = bass.ds(i * LCF + h * LC, LC)
                    ps80 = mmtile(80, LC, "ps80")
                    nc.tensor.matmul(ps80[:], xpwTA[:], xi2A[:, sl],
                                     start=True, stop=False)
                    nc.tensor.matmul(ps80[:], xpwTB[:], xi2B[:, sl],
                                     start=False, stop=True)
                    nc.scalar.copy(dt_c[:, ts(h, LC)], ps80[0:RK, :])
                    nc.scalar.copy(bm_c[:, ts(h, LC)], ps80[32:32 + S, :])
                    nc.scalar.copy(cm_c[:, ts(h, LC)], ps80[64:64 + S, :])
                return run

            def f_delta():
                dt_c = cx["dt_c"]
                deltaA = pT.tile([DA, LCF], bf16, name="deltaA")
                deltaB = pT.tile([DB, LCF], bf16, name="deltaB")
                for h in range(2):
                    dtpA = mmtile(DA, LC, "dtpA")
                    nc.tensor.matmul(dtpA[:], dtwT[:, 0:DA],
                                     dt_c[0:RK, ts(h, LC)],
                                     start=True, stop=True)
                    nc.scalar.activation(deltaA[:, ts(h, LC)], dtpA[:], AF.Exp,
                                         bias=dtbA[:])
                    dtpB = mmtile(DB, LC, "dtpB")
                    nc.tensor.matmul(dtpB[:], dtwT[:, DA:D],
                                     dt_c[0:RK, ts(h, LC)],
                                     start=True, stop=True)
                    nc.scalar.activation(deltaB[:, ts(h, LC)], dtpB[:], AF.Exp,
                                         bias=dtbB[:])
                nc.scalar.activation(deltaA[:], deltaA[:], AF.Ln, bias=1.0)
                nc.scalar.activation(deltaB[:], deltaB[:], AF.Ln, bias=1.0)
                cx["deltaA"], cx["deltaB"] = deltaA, deltaB

            def f_dx():
                deltaA, deltaB = cx["deltaA"], cx["deltaB"]
                dXA = pT.tile([DA, LCF], bf16, name="dXA")
                nc.vector.tensor_tensor(dXA[:], deltaA[:],
                                        xi2A[:, ts(i, LCF)], op=OP.mult)
                dXB = pT.tile([DB, LCF], bf16, name="dXB")
                nc.vector.tensor_tensor(dXB[:], deltaB[:],
                                        xi2B[:, ts(i, LCF)], op=OP.mult)
                cx["dXA"], cx["dXB"] = dXA, dXB

            def f_bc():
                bm_c, cm_c = cx["bm_c"], cx["cm_c"]
                brep = pT.tile([128, LCF], bf16, name="brep")
                crep = pT.tile([128, LCF], bf16, name="crep")
                for h in range(2):
                    brep_ps = mmtile(128, LC, "brep_ps")
                    nc.tensor.matmul(brep_ps[:], s01[:], bm_c[:, ts(h, LC)],
                                     start=True, stop=True)
                    nc.any.tensor_copy(brep[:, ts(h, LC)], brep_ps[:])
                    crep_ps = mmtile(128, LC, "crep_ps")
                    nc.tensor.matmul(crep_ps[:], s01[:], cm_c[:, ts(h, LC)],
                                     start=True, stop=True)
                    nc.any.tensor_copy(crep[:, ts(h, LC)], crep_ps[:])
                cx["brep"], cx["crep"] = brep, crep

            return [f_norm, f_trans, f_ip(0), f_ip(1), f_xp(0), f_xp(1),
                    f_delta, f_dx, f_bc]

        def emit_body(i, j, defer):
            """One scan tile; returns the y-matmul closure if defer."""
            cx = ctxs[i]
            deltaA, deltaB = cx["deltaA"], cx["deltaB"]
            dXA, dXB = cx["dXA"], cx["dXB"]
            brep, crep = cx["brep"], cx["crep"]
            jj = j if j < 16 else j - 16
            if j < 8:
                dsl, xsl = deltaA[0:64, :], dXA[0:64, :]
                psel = p0164[0:64, ts(jj % 8, 128)]
            elif j < 16:
                dsl, xsl = deltaA[64:128, :], dXA[64:128, :]
                psel = p0164[64:128, ts(jj % 8, 128)]
            else:
                dsl, xsl = deltaB[:, :], dXB[:, :]
                psel = p0164[0:64, ts(jj % 8, 128)]
            if j == 0:
                cx["ypsA0"] = ppy.tile([128, LC], f32, name="ypsA0",
                                       tag="ypsA0")
                cx["ypsA1"] = ppy.tile([128, LC], f32, name="ypsA1",
                                       tag="ypsA1")
                cx["ypsBp"] = ppy.tile([128, LC], f32, name="ypsBp",
                                       tag="ypsBp")
            dA_t = pT.tile([128, LCF], bf16, name="dA_t")
            dxc = pT.tile([128, LCF], bf16, name="dxc")
            for h in range(2):
                drep = mmtile(128, LC, "drep")
                nc.tensor.matmul(drep[:], psel, dsl[:, bass.ds(h * LC, LC)],
                                 start=True, stop=True)
                nc.scalar.activation(dA_t[:, ts(h, LC)], drep[:], AF.Exp,
                                     scale=acol[:, j:j + 1])
                dxrep = mmtile(128, LC, "dxrep")
                nc.tensor.matmul(dxrep[:], psel, xsl[:, bass.ds(h * LC, LC)],
                                 start=True, stop=True)
                nc.scalar.activation(dxc[:, ts(h, LC)], dxrep[:], AF.Copy)
            dBu = pT.tile([128, LCF], bf16, name="dBu")
            nc.vector.tensor_tensor(dBu[:], dxc[:], brep[:], op=OP.mult)
            h_t = pT.tile([128, LCF], bf16, name="h_t")
            init = 0.0 if i == 0 else hlast[:, j:j + 1]
            nc.vector.tensor_tensor_scan(
                h_t[:], dA_t[:], dBu[:], init, op0=OP.mult, op1=OP.add)
            nc.vector.tensor_copy(hlast[:, j:j + 1], h_t[:, LCF - 1:LCF])
            ch = pT.tile([128, LCF], bf16, name="ch")
            nc.vector.tensor_tensor(ch[:], h_t[:], crep[:], op=OP.mult)

            def y_mm():
                if j < 16:
                    nc.tensor.matmul(cx["ypsA0"][:], r01all[:, ts(jj, 128)],
                                     ch[:, 0:LC], start=(j == 0), stop=False)
                    nc.tensor.matmul(cx["ypsA1"][:], r01all[:, ts(jj, 128)],
                                     ch[:, LC:LCF], start=(j == 0), stop=False)
                else:
                    nc.tensor.matmul(cx["ypsBp"][0:DB, :],
                                     r01ball[:, ts(jj, 64)],
                                     ch[:, 0:LC], start=(j == 16), stop=False,
                                     skip_group_check=True)
                    nc.tensor.matmul(cx["ypsBp"][DB:128, :],
                                     r01ball[:, ts(jj, 64)],
                                     ch[:, LC:LCF], start=(j == 16),
                                     stop=False, skip_group_check=True)
            if defer:
                return y_mm
            y_mm()
            return None

        def emit_tail(i):
            cx = ctxs[i]
            ypsA0, ypsA1 = cx["ypsA0"], cx["ypsA1"]
            ypsB0 = cx["ypsBp"][0:DB, :]
            ypsB1 = cx["ypsBp"][DB:128, :]
            # Dp residual folded into the y accumulation as a diagonal matmul
            yA = pT.tile([DA, LCF], bf16, name="yA", bufs=1)
            yB = pT.tile([DB, LCF], bf16, name="yB", bufs=1)
            for h, (ya_ps, yb_ps) in enumerate([(ypsA0[:], ypsB0),
                                                (ypsA1[:], ypsB1)]):
                sl = bass.ds(i * LCF + h * LC, LC)
                nc.tensor.matmul(ya_ps, dpdA[:], xi2A[:, sl],
                                 start=False, stop=True, skip_group_check=True)
                nc.tensor.matmul(yb_ps, dpdB[:], xi2B[:, sl],
                                 start=False, stop=True, skip_group_check=True)
                nc.vector.tensor_tensor(yA[:, ts(h, LC)], ya_ps,
                                        zsA[:, sl], op=OP.mult)
                nc.vector.tensor_tensor(yB[:, ts(h, LC)], yb_ps,
                                        zsB[:, sl], op=OP.mult)
            mout = pT.tile([C, LCF], bf16, name="mout", bufs=1)
            for h in range(2):
                mps = mmtile(C, LC, "mps")
                nc.tensor.matmul(mps[:], outwTA[:], yA[:, ts(h, LC)],
                                 start=True, stop=False)
                nc.tensor.matmul(mps[:], outwTB[:], yB[:, ts(h, LC)],
                                 start=False, stop=True)
                nc.scalar.copy(mout[:, ts(h, LC)], mps[:])
            # Z = xdq + mout^T computed wholly in PSUM
            for r0 in range(0, 16, 4):
                zps = pmm.tile([WQ, 4 * C], f32, name="zps", tag="mm")
                zps3 = zps.rearrange("p (r c) -> p r c", r=4)
                for k in range(4):
                    r = r0 + k
                    nc.tensor.matmul(zps3[:, k, :], mout[:, ts(r, WQ)],
                                     identb[:], start=True, stop=False)
                    nc.tensor.matmul(zps3[:, k, :], identb[0:WQ, 0:WQ],
                                     xdq3[:, :, 16 * i + r],
                                     start=False, stop=True)
                hq0 = 16 * i + r0
                nc.scalar.activation(Z3[:, :, hq0:hq0 + 4],
                                     zps3.transpose([0, 2, 1]), AF.Copy)

        fronts = [make_front(i) for i in range(NI)]
        for f in fronts[0]:
            f()
        # drain schedule clusters the two silu-bearing closures (f_ip) at one
        # slot to limit Exp<->Silu activation-table swaps on Scalar
        drains = {1: 2, 3: 2, 5: 2, 7: 1, 9: 1, 11: 1}
        start_j = 0
        for i in range(NI):
            pending = list(fronts[i + 1]) if i + 1 < NI else []
            for j in range(start_j, NT):
                emit_body(i, j, defer=False)
                for _ in range(drains.get(j, 0)):
                    if pending:
                        pending.pop(0)()
            while pending:
                pending.pop(0)()
            # overlap this chunk's PE-heavy tail with the next chunk's first
            # scans: emit 3 bodies (y-matmuls deferred until the tail has
            # consumed this chunk's y psums)
            if i + 1 < NI:
                deferred = [emit_body(i + 1, jd, defer=True) for jd in range(3)]
                start_j = 3
            else:
                deferred = []
            emit_tail(i)
            for fn in deferred:
                fn()
        pT.release()
        pC.release()
        pE.release()
        pG.release()

        # =============== Phase G: IDCT contribution ===============
        pH = tc.alloc_tile_pool(name="pH", bufs=1)
        # t7[hq, c, W] = sum_wq Z[wq, c, hq] * Mw_q[wq, W]  (per-c matmul)
        t7 = pH.tile([HQ, C * W], bf16)
        t7_3 = t7.rearrange("p (c w) -> p c w", c=C)
        for ci, c0 in enumerate(range(0, C, 4)):
            t7ps = tptile(HQ, 4 * W, "t7ps")
            t7ps3 = t7ps.rearrange("p (c w) -> p c w", c=4)
            for k in range(4):
                nc.tensor.matmul(t7ps3[:, k, :], Z3[:, c0 + k, :],
                                 mwq64[64:128, :], start=True, stop=True)
            if ci % 2 == 0:
                nc.scalar.activation(t7_3[:, c0:c0 + 4, :], t7ps3[:, :, :],
                                     AF.Copy)
            else:
                nc.vector.tensor_copy(t7_3[:, c0:c0 + 4, :], t7ps3[:, :, :])
        ctr_h = contrib
        pSo = tc.alloc_tile_pool(name="pSo", bufs=3)
        for i in range(24):
            cps = mmtile(H, LC, "cps")
            nc.tensor.matmul(cps[:], mhq[:], t7[:, ts(i, LC)], start=True, stop=True)
            csb = pSo.tile([H, LC], bf16, name="csb")
            if i % 2 == 0:
                nc.scalar.activation(csb[:], cps[:], AF.Copy)
            else:
                nc.vector.tensor_copy(csb[:], cps[:])
            nc.sync.dma_start(
                ctr_h[:, ts(i, 4), :],
                csb[:].rearrange("h (c w) -> h c w", c=4))
        pSo.release()
        pH.release()
        pD_.release()
        ptp.release()
        ppy.release()
        pmm.release()
        consts.release()

    nc.compile()
    return nc


def _host_inputs(inputs):
    """Build the 8 per-core input maps."""
    x = inputs["x"]
    ln_w, ln_b = inputs["ln_w"], inputs["ln_b"]
    Mh = _dct_mat(H)
    Mw = _dct_mat(W)
    ident = np.eye(128, dtype=np.float32)
    # within-tile partition order p = 8*s + r (r = d offset, s = state):
    # makes row replication p -> p%8 a contiguous-partition doubling
    s01 = np.zeros((S, 128), np.float32)
    p0164 = np.zeros((128, 128 * 8), np.float32)
    r01all = np.zeros((128, 128 * 16), np.float32)
    r01ball = np.zeros((128, 64 * 8), np.float32)
    for p in range(128):
        s01[p // 8, p] = 1.0
        for j in range(8):
            p0164[8 * j + p % 8, 128 * j + p] = 1.0
            p0164[64 + 8 * j + p % 8, 128 * j + p] = 1.0
        for j in range(16):
            r01all[p, 128 * j + 8 * j + p % 8] = 1.0
        for j in range(8):
            r01ball[p, 64 * j + 8 * j + p % 8] = 1.0
    in_maps = []
    for k in range(8):
        b, q = k // 4, k % 4
        h0 = (q // 2) * HQ
        w0 = (q % 2) * WQ
        in_w2 = (inputs["in_w"][q] * ln_w[None, :]).astype(np.float32)
        bias_e = (inputs["in_w"][q] @ ln_b).astype(np.float32)
        # conv folded into in_proj: tap k' applies weight conv_w[:, 3-k'] to
        # positions shifted back by k'
        convw = inputs["conv_w"][q].astype(np.float32)        # [D, 4]
        convb = inputs["conv_b"][q].astype(np.float32)        # [D]
        w_rev = convw[:, ::-1]
        WX = in_w2[:D]                                        # [D, C]
        inwtap = np.concatenate(
            [(WX * w_rev[:, kk][:, None]).T for kk in range(KCONV)], axis=1)
        bconv = bias_e[:D] * convw.sum(1) + convb
        bc3 = np.stack(
            [bias_e[:D] * w_rev[:, :t + 1].sum(1) + convb
             for t in range(KCONV - 1)], axis=1)              # [D, 3]
        xpw80 = np.zeros((D, 80), np.float32)
        xpwT = inputs["xp_w"][q].T
        xpw80[:, 0:RK] = xpwT[:, 0:RK]
        xpw80[:, 32:32 + S] = xpwT[:, RK:RK + S]
        xpw80[:, 64:64 + S] = xpwT[:, RK + S:RK + 2 * S]
        A = (-np.exp(inputs["A_log"][q])).astype(np.float32)  # [D, S]
        acol = np.zeros((128, NT), np.float32)
        for j in range(NT):
            for p in range(128):
                acol[p, j] = A[j * 8 + p % 8, p // 8]
        m = {
            "xbh": np.ascontiguousarray(x[b].transpose(1, 0, 2)),
            "mhqT": Mh[h0:h0 + HQ, :].T,
            "mwqT": Mw[w0:w0 + WQ, :].T,
            "mhq": Mh[h0:h0 + HQ, :],
            "mwq": Mw[w0:w0 + WQ, :],
            "ident": ident,
            "inwtap": inwtap,
            "inwz": in_w2[D:].T,
            "biasz": bias_e[D:, None],
            "bconv": bconv[:, None],
            "bc3": bc3,
            "xpwT": xpw80,
            "dtwT": inputs["dt_w"][q].T,
            "dtb": inputs["dt_b"][q][:, None],
            "acol": acol,
            "dpdA": np.diag(inputs["Dp"][q][:DA]),
            "dpdB": np.diag(inputs["Dp"][q][DA:]),
            "outwT": inputs["out_w"][q].T,
            "p0164": p0164,
            "s01": s01,
            "r01all": r01all,
            "r01ball": r01ball,
        }
        import ml_dtypes
        bf = ["inwtap", "inwz", "xpwT", "dtwT", "outwT", "p0164", "s01",
              "dpdA", "dpdB",
              "r01all", "r01ball", "xbh", "mhqT", "mwqT", "mhq", "mwq",
              "contrib"]
        in_maps.append({
            kk: np.ascontiguousarray(np.asarray(
                vv, ml_dtypes.bfloat16 if kk in bf else np.float32))
            for kk, vv in m.items()})
    return in_maps


def kernel(**inputs):
    from concourse import bass_utils
    inputs = {k: np.asarray(v) for k, v in inputs.items()}
    if "nc" not in _BUILT:
        _BUILT["nc"] = _build_nc()
    nc = _BUILT["nc"]
    in_maps = _host_inputs(inputs)
    res = bass_utils.run_bass_kernel_spmd(nc, in_maps, core_ids=list(range(8)))
    out = np.zeros((B, C, H, W), np.float32)
    for k in range(8):
        out[k // 4] += np.asarray(
            res.results[k]["contrib"], np.float32).transpose(1, 0, 2)
    return out


if __name__ == "__main__":
    # smoke: random inputs, shape check only
    rng = np.random.default_rng(0)
    demo = {
        "x": rng.standard_normal((B, C, H, W), np.float32),
        "ln_w": np.ones(C, np.float32), "ln_b": np.zeros(C, np.float32),
        "in_w": rng.standard_normal((4, 2 * D, C), np.float32) * 0.02,
        "conv_w": rng.standard_normal((4, D, KCONV), np.float32) * 0.02,
        "conv_b": np.zeros((4, D), np.float32),
        "xp_w": rng.standard_normal((4, RK + 2 * S, D), np.float32) * 0.02,
        "dt_w": rng.standard_normal((4, D, RK), np.float32) * 0.02,
        "dt_b": np.full((4, D), -4.0, np.float32),
        "A_log": np.tile(np.log(np.arange(1, S + 1, dtype=np.float32)), (4, D, 1)),
        "Dp": np.ones((4, D), np.float32),
        "out_w": rng.standard_normal((4, C, D), np.float32) * 0.02,
    }
    out = kernel(**demo)
    print("kernel output:", out.shape, out.dtype)



# revision 37
# speedup vs baseline: 1.1363x; 1.1363x over previous
"""Trainium2 Bass kernel for nn_EnhancedFreqLCBlock.

Self-contained: accepts FULL inputs, returns FULL output.
Sharding: 8 cores = 2 batches x 4 quadrant Mamba blocks (expert parallel).
Each core: mask -> quadrant 2D-DCT -> channel LN -> Mamba (hardware
tensor_tensor_scan recurrence) -> residual -> quadrant IDCT contribution.
Host sums the 4 quadrant contributions per batch.
"""
import numpy as np

B, C, H, W = 2, 96, 128, 128
HQ, WQ = H // 2, W // 2          # 64, 64
L = HQ * WQ                      # 4096
D = 192                          # d_inner
S = 16                           # d_state
RK = 6                           # dt_rank
KCONV = 4
NCHUNK = 8
LC = L // NCHUNK                 # 512
NT = (D * S) // 128              # 24 scan partition-tiles
DA, DB = 128, 64                 # d split 192 = 128 + 64

_BUILT = {}


def _dct_mat(N):
    n = np.arange(N)
    M = np.cos(np.pi * (2 * n[None, :] + 1) * n[:, None] / (2 * N)) * np.sqrt(2.0 / N)
    M[0] *= 1.0 / np.sqrt(2.0)
    return M.astype(np.float32)


def _build_nc():
    import concourse.bacc as bacc
    import concourse.bass as bass
    import concourse.mybir as mybir
    import concourse.tile as tile

    f32 = mybir.dt.float32
    bf16 = mybir.dt.bfloat16
    AF = mybir.ActivationFunctionType
    OP = mybir.AluOpType
    AX = mybir.AxisListType
    ts = bass.ts

    nc = bacc.Bacc()

    # ---------------- DRAM I/O ----------------
    xbh = nc.dram_tensor("xbh", [H, C, W], bf16, kind="ExternalInput")
    d_mhqT = nc.dram_tensor("mhqT", [H, HQ], bf16, kind="ExternalInput")
    d_mwqT = nc.dram_tensor("mwqT", [W, WQ], bf16, kind="ExternalInput")
    d_mhq = nc.dram_tensor("mhq", [HQ, H], bf16, kind="ExternalInput")
    d_mwq = nc.dram_tensor("mwq", [WQ, W], bf16, kind="ExternalInput")
    d_ident = nc.dram_tensor("ident", [128, 128], f32, kind="ExternalInput")
    d_inwtap = nc.dram_tensor("inwtap", [C, KCONV * D], bf16,
                              kind="ExternalInput")
    d_inwz = nc.dram_tensor("inwz", [C, D], bf16, kind="ExternalInput")
    d_biasz = nc.dram_tensor("biasz", [D, 1], f32, kind="ExternalInput")
    d_bconv = nc.dram_tensor("bconv", [D, 1], f32, kind="ExternalInput")
    d_bc3 = nc.dram_tensor("bc3", [D, KCONV - 1], f32, kind="ExternalInput")
    d_xpwT = nc.dram_tensor("xpwT", [D, 80], bf16, kind="ExternalInput")
    d_dtwT = nc.dram_tensor("dtwT", [RK, D], bf16, kind="ExternalInput")
    d_dtb = nc.dram_tensor("dtb", [D, 1], f32, kind="ExternalInput")
    d_acol = nc.dram_tensor("acol", [128, NT], f32, kind="ExternalInput")
    d_dpdA = nc.dram_tensor("dpdA", [DA, DA], bf16, kind="ExternalInput")
    d_dpdB = nc.dram_tensor("dpdB", [DB, DB], bf16, kind="ExternalInput")
    d_outwT = nc.dram_tensor("outwT", [D, C], bf16, kind="ExternalInput")
    d_p0164 = nc.dram_tensor("p0164", [128, 128 * 8], bf16, kind="ExternalInput")
    d_s01 = nc.dram_tensor("s01", [S, 128], bf16, kind="ExternalInput")
    d_r01all = nc.dram_tensor("r01all", [128, 128 * 16], bf16, kind="ExternalInput")
    d_r01ball = nc.dram_tensor("r01ball", [128, 64 * 8], bf16, kind="ExternalInput")
    contrib = nc.dram_tensor("contrib", [H, C, W], bf16, kind="ExternalOutput")
    # per-chunk DRAM scratch for the dX row-replication round-trip
    d_dxs = nc.dram_tensor("dxscratch", [NCHUNK // 2, D, 2 * LC], bf16,
                           kind="Internal")

    with tile.TileContext(nc) as tc:
        consts = tc.alloc_tile_pool(name="consts", bufs=1)
        # issue the big input loads before the ~30 const loads: SP
        # dispatches DMAs in program order and the mask path gates startup
        pD_ = tc.alloc_tile_pool(name="pD", bufs=1)
        pB = tc.alloc_tile_pool(name="pB", bufs=1)
        pXH = tc.alloc_tile_pool(name="pXH", bufs=1)
        pA = tc.alloc_tile_pool(name="pA", bufs=1)
        # center row of x (h = H/2): one contiguous descriptor, issued first
        crow = pA.tile([1, C * W], bf16)
        nc.sync.dma_start(crow[:], xbh[H // 2:H // 2 + 1, :, :])
        xh = pXH.tile([H, C * W], bf16)
        xh3 = xh.rearrange("h (c w) -> h c w", c=C)
        # c-chunked loads keep full 128-partition spans (4x the DMA rate of
        # h-chunked loads); x is loaded only once (no c-major copy)
        for i in range(4):
            nc.sync.dma_start(xh3[:, ts(i, 24), :], xbh[:, ts(i, 24), :])

        def cload(dram, shape, dt=f32):
            t = consts.tile(shape, dt, name=f"c_{dram.name}")
            nc.sync.dma_start(t[:], dram[:])
            return t

        def cload2(dram, dt=f32):
            ta = consts.tile([DA] + list(dram.shape[1:]), dt, name=f"cA_{dram.name}")
            nc.sync.dma_start(ta[:], dram[0:DA])
            tb = consts.tile([DB] + list(dram.shape[1:]), dt, name=f"cB_{dram.name}")
            nc.sync.dma_start(tb[:], dram[DA:D])
            return ta, tb

        mhqT = cload(d_mhqT, [H, HQ], bf16)
        mwqT = cload(d_mwqT, [W, WQ], bf16)
        mhq = cload(d_mhq, [HQ, H], bf16)
        mwq64 = consts.tile([128, W], bf16, name="c_mwq64")
        nc.sync.dma_start(mwq64[64:128, :], d_mwq[:])
        ident = cload(d_ident, [128, 128])
        identb = consts.tile([C, C], bf16, name="identb")
        nc.vector.tensor_copy(identb[:], ident[0:C, 0:C])
        inwtap = cload(d_inwtap, [C, KCONV * D], bf16)
        inwz = cload(d_inwz, [C, D], bf16)
        biaszA, biaszB = cload2(d_biasz)
        bconvA, bconvB = cload2(d_bconv)
        bc3A, bc3B = cload2(d_bc3)
        xpwTA, xpwTB = cload2(d_xpwT, bf16)
        dtwT = cload(d_dtwT, [RK, D], bf16)
        dtbA, dtbB = cload2(d_dtb)
        acol = cload(d_acol, [128, NT])
        dpdA = cload(d_dpdA, [DA, DA], bf16)
        dpdB = cload(d_dpdB, [DB, DB], bf16)
        outwTA, outwTB = cload2(d_outwT, bf16)
        p0164 = cload(d_p0164, [128, 128 * 8], bf16)
        s01 = cload(d_s01, [S, 128], bf16)
        r01all = cload(d_r01all, [128, 128 * 16], bf16)
        r01ball = cload(d_r01ball, [128, 64 * 8], bf16)
        onesr = consts.tile([1, 128], f32)
        nc.vector.memset(onesr[:], 1.0)
        onesrb = consts.tile([1, 128], bf16)
        nc.vector.memset(onesrb[:], 1.0)
        ones96b = consts.tile([C, 1], bf16)
        nc.vector.memset(ones96b[:], 1.0)
        eps64 = consts.tile([WQ, 1], f32)
        nc.vector.memset(eps64[:], 1e-5)

        # persistent psum pools (8 banks total: 4 + 2 + 2)
        pmm = tc.alloc_tile_pool(name="pmm", bufs=4, space="PSUM")
        ppy = tc.alloc_tile_pool(name="ppy", bufs=1, space="PSUM")
        ptp = tc.alloc_tile_pool(name="ptp", bufs=1, space="PSUM")

        def mmtile(p, n, nm):
            return pmm.tile([p, n], f32, name=nm, tag="mm")

        def tptile(p, n, nm, dt=f32):
            return ptp.tile([p, n], dt, name=nm, tag="tp")

        # =============== Phase A: mask ===============
        # center pixel vector from the crow strip (strided view, stride W)
        crow3 = crow.rearrange("p (c w) -> p c w", c=C)
        center = crow3[0:1, :, W // 2]                      # [1, C]
        csq = pA.tile([1, C], f32)
        nc.vector.tensor_tensor(csq[:], center, center, op=OP.mult)
        cn = pA.tile([1, 1], f32)
        nc.vector.tensor_reduce(cn[:], csq[:], axis=AX.X, op=OP.add)
        s049 = pA.tile([1, 1], f32)
        nc.vector.tensor_scalar_mul(s049[:], cn[:], 0.49)
        s049p = tptile(128, 1, "s049p")
        nc.tensor.matmul(s049p[:], onesr[:], s049[:], start=True, stop=True)
        s049b = pA.tile([128, 1], f32)
        nc.vector.tensor_copy(s049b[:], s049p[:])
        # center replicated to all 128 partitions
        cenp = tptile(128, C, "cenp")
        nc.tensor.matmul(cenp[:], onesrb[:], center, start=True, stop=True)
        cenb = pA.tile([128, C], bf16)
        nc.vector.tensor_copy(cenb[:], cenp[:])

        num_hw = pA.tile([128, 128], f32)
        ssq_hw = pA.tile([128, 128], f32)
        # per c-chunk (matching the 4 xh3 loads): num = sum_c x*center via an
        # inner-broadcast multiply + middle-axis reduce; ssq = sum_c x^2
        pSq = tc.alloc_tile_pool(name="pSq", bufs=2)
        for g in range(4):
            csl = bass.ds(24 * g, 24)
            # write products/squares w-major so the c-reduce runs on a
            # contiguous innermost axis (strided reduces run at half rate)
            prod = pSq.tile([128, 128 * 24], bf16, name="prod")
            prodT = prod.rearrange("h (w c) -> h w c", c=24)
            nc.vector.tensor_tensor(
                prodT[:, :, :], xh3[:, csl, :].transpose([0, 2, 1]),
                cenb[:, None, csl].broadcast_to([128, 128, 24]), op=OP.mult)
            pn = pSq.tile([128, 128], f32, name="pn")
            nc.vector.tensor_reduce(pn[:], prodT[:, :, :], axis=AX.X, op=OP.add)
            sq = pSq.tile([128, 128 * 24], bf16, name="sq")
            sqT = sq.rearrange("h (w c) -> h w c", c=24)
            nc.scalar.activation(sqT[:, :, :], xh3[:, csl, :].transpose([0, 2, 1]),
                                 AF.Square)
            psq_ = pSq.tile([128, 128], f32, name="psq_")
            nc.vector.tensor_reduce(psq_[:], sqT[:, :, :], axis=AX.X, op=OP.add)
            if g == 0:
                nc.vector.tensor_copy(num_hw[:], pn[:])
                nc.vector.tensor_copy(ssq_hw[:], psq_[:])
            else:
                nc.vector.tensor_tensor(num_hw[:], num_hw[:], pn[:], op=OP.add)
                nc.vector.tensor_tensor(ssq_hw[:], ssq_hw[:], psq_[:],
                                        op=OP.add)
        pSq.release()

        thr = pA.tile([128, 128], f32)
        nc.scalar.activation(thr[:], ssq_hw[:], AF.Sqrt, bias=0.0, scale=s049b[:])
        nc.vector.tensor_scalar_add(thr[:], thr[:], 0.7e-6)
        mask_hw = pA.tile([128, 128], bf16)
        nc.vector.tensor_tensor(mask_hw[:], num_hw[:], thr[:], op=OP.is_ge)
        for i in range(4):
            nc.vector.tensor_tensor(
                 xh3[:, ts(i, 24), :], xh3[:, ts(i, 24), :],
                 mask_hw[:, None, :].broadcast_to([128, 24, 128]), op=OP.mult)
        pA.release()

        # =============== Phase B: forward DCT ===============
        # t2[w, c, hq] = sum_h x[h, c, w] * Mh_q[hq, h]  (per-c matmul, no
        # separate transpose pass)
        t2 = pB.tile([W, C * HQ], bf16)
        t2_3 = t2.rearrange("p (c q) -> p c q", c=C)
        for c0 in range(0, C, 8):
            tps = tptile(W, 8 * HQ, "tps")
            tps3 = tps.rearrange("p (c q) -> p c q", c=8)
            for k in range(8):
                nc.tensor.matmul(tps3[:, k, :], xh3[:, c0 + k, :], mhqT[:],
                                 start=True, stop=True)
            nc.scalar.activation(t2_3[:, c0:c0 + 8, :], tps3[:, :, :], AF.Copy)
        pXH.release()

        # xdqZ: rows 0:64 = xdq (base 0 for DVE pairing), rows 64:128 = Z
        xdqZ = pD_.tile([128, C * HQ], bf16)
        xdq3 = xdqZ.rearrange("p (c q) -> p c q", c=C)[0:HQ, :, :]
        Z3 = xdqZ.rearrange("p (c q) -> p c q", c=C)[HQ:128, :, :]
        xdq2 = xdqZ[0:HQ, :]
        Z2 = xdqZ[HQ:128, :]
        for i in range(12):
            xps = mmtile(WQ, LC, "xps")
            nc.tensor.matmul(xps[:], mwqT[:], t2[:, ts(i, LC)], start=True, stop=True)
            nc.any.tensor_copy(xdq2[:, ts(i, LC)], xps[:])
        pB.release()

        # =============== Phase C: LayerNorm over c ===============
        pG = tc.alloc_tile_pool(name="pG", bufs=1)
        pE = tc.alloc_tile_pool(name="pE", bufs=1)
        pC = tc.alloc_tile_pool(name="pC", bufs=1)
        # LN stats chunked to the 12 stage-2 copy chunks (8 c's each) so the
        # reduces overlap the DCT matmuls instead of serializing the machine
        smu = pC.tile([WQ, HQ], f32)
        ssq2 = pC.tile([WQ, HQ], f32)
        xn = pC.tile([WQ, C * HQ], bf16)  # first used as xdq^2 scratch
        xn3s = xn.rearrange("p (c q) -> p c q", c=C)
        pPart = tc.alloc_tile_pool(name="pPart", bufs=3)
        for cb in range(C // 8):
            csl = bass.ds(cb * 8, 8)
            nc.vector.tensor_tensor(xn3s[:, csl, :], xdq3[:, csl, :],
                                    xdq3[:, csl, :], op=OP.mult)
            pm = pPart.tile([WQ, HQ], f32, name="pm")
            nc.vector.tensor_reduce(
                pm[:], xdq3[:, csl, :].transpose([0, 2, 1]), axis=AX.X,
                op=OP.add)
            psq = pPart.tile([WQ, HQ], f32, name="psq")
            nc.vector.tensor_reduce(
                psq[:], xn3s[:, csl, :].transpose([0, 2, 1]), axis=AX.X,
                op=OP.add)
            if cb == 0:
                nc.vector.tensor_copy(smu[:], pm[:])
                nc.vector.tensor_copy(ssq2[:], psq[:])
            else:
                nc.vector.tensor_tensor(smu[:], smu[:], pm[:], op=OP.add)
                nc.vector.tensor_tensor(ssq2[:], ssq2[:], psq[:], op=OP.add)
        pPart.release()
        mu = pC.tile([WQ, HQ], f32)
        nc.vector.tensor_scalar_mul(mu[:], smu[:], 1.0 / C)
        var = pC.tile([WQ, HQ], f32)
        nc.vector.tensor_scalar_mul(ssq2[:], ssq2[:], 1.0 / C)
        nc.vector.tensor_tensor(var[:], mu[:], mu[:], op=OP.mult)
        nc.vector.tensor_tensor(var[:], ssq2[:], var[:], op=OP.subtract)
        sd = pC.tile([WQ, HQ], f32)
        nc.scalar.activation(sd[:], var[:], AF.Sqrt, bias=eps64[:])
        inv = pC.tile([WQ, HQ], f32)
        nc.vector.reciprocal(inv[:], sd[:])
        # bf16 stats so the normalize runs at DVE 2x rate; chunk by hq so
        # the transposes start before the whole tensor is normalized
        mub = pC.tile([WQ, HQ], bf16)
        nc.vector.tensor_copy(mub[:], mu[:])
        invb = pC.tile([WQ, HQ], bf16)
        nc.vector.tensor_copy(invb[:], inv[:])
        xn3 = xn.rearrange("p (c q) -> p c q", c=C)
        xn_c = pE.tile([C, L], bf16)
        xi2A = pG.tile([DA, L], bf16)
        zsA = pG.tile([DA, L], bf16)
        xi2B_t = pG.tile([DB, L], bf16, name="xi2B_t")
        zsB_t = pG.tile([DB, L], bf16, name="zsB_t")
        xi2B = xi2B_t[:, :]
        zsB = zsB_t[:, :]

        # ====== fused loop: LN-normalize -> in_proj(+conv taps) -> scan ======
        # (per 1024-col chunk so PE/Scalar front-end work overlaps the
        # DVE-bound scan of the previous chunk)
        pT = tc.alloc_tile_pool(name="pT", bufs=3)
        hlast = pG.tile([128, NT], bf16)
        LCF = 2 * LC
        NI = NCHUNK // 2
        ctxs = [dict() for _ in range(NI)]

        def make_front(i):
            """Front-end of chunk i as a closure list, so it can be emitted
            interleaved into chunk i-1's scan loop."""
            cx = ctxs[i]
            h0 = 16 * i

            def f_norm():
                hsl = bass.ds(h0, 16)
                nc.vector.tensor_tensor(
                    xn3[:, :, hsl], xdq3[:, :, hsl],
                    mub[:, None, hsl].broadcast_to([WQ, C, 16]),
                    op=OP.subtract)
                nc.vector.tensor_tensor(
                    xn3[:, :, hsl], xn3[:, :, hsl],
                    invb[:, None, hsl].broadcast_to([WQ, C, 16]), op=OP.mult)

            def f_trans():
                tps2 = tptile(C, 16 * WQ, "tps2", bf16)
                tps2_3 = tps2.rearrange("p (h q) -> p h q", h=16)
                for k in range(16):
                    nc.tensor.matmul(tps2_3[:, k, :], xn3[:, :, h0 + k],
                                     identb[0:WQ, 0:WQ],
                                     is_transpose=True, start=True, stop=True)
                nc.scalar.activation(xn_c[:, h0 * WQ:(h0 + 16) * WQ], tps2[:],
                                     AF.Copy)

            def f_ip(c2):
                def run():
                    c8 = 2 * i + c2
                    t0 = c8 * LC
                    psA = mmtile(128, LC, "psA")
                    psBz = mmtile(128, LC, "psBz")
                    psZ = mmtile(128, LC, "psZ")
                    for k in range(KCONV):
                        if c8 == 0:
                            rhs = xn_c[:, 0:LC - k]
                            oA = psA[:, k:LC]
                            oB = psBz[0:64, k:LC]
                        else:
                            rhs = xn_c[:, t0 - k:t0 - k + LC]
                            oA = psA[:, :]
                            oB = psBz[0:64, :]
                        nc.tensor.matmul(oA, inwtap[:, k * D:k * D + DA], rhs,
                                         start=(k == 0), stop=(k == KCONV - 1))
                        nc.tensor.matmul(oB, inwtap[:, k * D + DA:(k + 1) * D],
                                         rhs, start=(k == 0),
                                         stop=(k == KCONV - 1),
                                         skip_group_check=True)
                    nc.tensor.matmul(psZ[:], inwz[:, 0:DA], xn_c[:, ts(c8, LC)],
                                     start=True, stop=True)
                    nc.tensor.matmul(psBz[64:128, :], inwz[:, DA:D],
                                     xn_c[:, ts(c8, LC)],
                                     start=True, stop=True,
                                     skip_group_check=True)
                    nc.scalar.activation(xi2A[:, ts(c8, LC)], psA[:], AF.Silu,
                                         bias=bconvA[:])
                    nc.scalar.activation(xi2B[:, ts(c8, LC)], psBz[0:64, :],
                                         AF.Silu, bias=bconvB[:])
                    nc.scalar.activation(zsA[:, ts(c8, LC)], psZ[:], AF.Silu,
                                         bias=biaszA[:])
                    nc.scalar.activation(zsB[:, ts(c8, LC)], psBz[64:128, :],
                                         AF.Silu, bias=biaszB[:])
                    if c8 == 0:
                        # first KCONV-1 cols: truncated tap sum -> redo silu
                        # with the truncated bias
                        for t in range(KCONV - 1):
                            nc.scalar.activation(
                                xi2A[:, t:t + 1], psA[:, t:t + 1], AF.Silu,
                                bias=bc3A[:, t:t + 1])
                            nc.scalar.activation(
                                xi2B[:, t:t + 1], psBz[0:64, t:t + 1], AF.Silu,
                                bias=bc3B[:, t:t + 1])
                return run

            def f_xp(h):
                def run():
                    if h == 0:
                        cx["dt_c"] = pT.tile([RK, LCF], bf16, name="dt_c",
                                             bufs=1)
                        cx["bm_c"] = pT.tile([S, LCF], bf16, name="bm_c",
                                             bufs=1)
                        cx["cm_c"] = pT.tile([S, LCF], bf16, name="cm_c",
                                             bufs=1)
                    dt_c, bm_c, cm_c = cx["dt_c"], cx["bm_c"], cx["cm_c"]
                    sl = bass.ds(i * LCF + h * LC, LC)
                    ps80 = mmtile(80, LC, "ps80")
                    nc.tensor.matmul(ps80[:], xpwTA[:], xi2A[:, sl],
                                     start=True, stop=False)
                    nc.tensor.matmul(ps80[:], xpwTB[:], xi2B[:, sl],
                                     start=False, stop=True)
                    nc.scalar.copy(dt_c[:, ts(h, LC)], ps80[0:RK, :])
                    nc.scalar.copy(bm_c[:, ts(h, LC)], ps80[32:32 + S, :])
                    nc.scalar.copy(cm_c[:, ts(h, LC)], ps80[64:64 + S, :])
                return run

            def f_delta():
                dt_c = cx["dt_c"]
                deltaA = pT.tile([DA, LCF], bf16, name="deltaA")
                deltaB = pT.tile([DB, LCF], bf16, name="deltaB")
                for h in range(2):
                    dtpA = mmtile(DA, LC, "dtpA")
                    nc.tensor.matmul(dtpA[:], dtwT[:, 0:DA],
                                     dt_c[0:RK, ts(h, LC)],
                                     start=True, stop=True)
                    nc.scalar.activation(deltaA[:, ts(h, LC)], dtpA[:], AF.Exp,
                                         bias=dtbA[:])
                    dtpB = mmtile(DB, LC, "dtpB")
                    nc.tensor.matmul(dtpB[:], dtwT[:, DA:D],
                                     dt_c[0:RK, ts(h, LC)],
                                     start=True, stop=True)
                    nc.scalar.activation(deltaB[:, ts(h, LC)], dtpB[:], AF.Exp,
                                         bias=dtbB[:])
                nc.scalar.activation(deltaA[:], deltaA[:], AF.Ln, bias=1.0)
                nc.scalar.activation(deltaB[:], deltaB[:], AF.Ln, bias=1.0)
                cx["deltaA"], cx["deltaB"] = deltaA, deltaB

            def f_dx():
                deltaA, deltaB = cx["deltaA"], cx["deltaB"]
                dXA = pT.tile([DA, LCF], bf16, name="dXA")
                nc.vector.tensor_tensor(dXA[:], deltaA[:],
                                        xi2A[:, ts(i, LCF)], op=OP.mult)
                dXB = pT.tile([DB, LCF], bf16, name="dXB")
                nc.vector.tensor_tensor(dXB[:], deltaB[:],
                                        xi2B[:, ts(i, LCF)], op=OP.mult)
                cx["dXA"], cx["dXB"] = dXA, dXB

            def f_bc():
                bm_c, cm_c = cx["bm_c"], cx["cm_c"]
                brep = pT.tile([128, LCF], bf16, name="brep")
                crep = pT.tile([128, LCF], bf16, name="crep")
                for h in range(2):
                    brep_ps = mmtile(128, LC, "brep_ps")
                    nc.tensor.matmul(brep_ps[:], s01[:], bm_c[:, ts(h, LC)],
                                     start=True, stop=True)
                    nc.any.tensor_copy(brep[:, ts(h, LC)], brep_ps[:])
                    crep_ps = mmtile(128, LC, "crep_ps")
                    nc.tensor.matmul(crep_ps[:], s01[:], cm_c[:, ts(h, LC)],
                                     start=True, stop=True)
                    nc.any.tensor_copy(crep[:, ts(h, LC)], crep_ps[:])
                cx["brep"], cx["crep"] = brep, crep

            return [f_norm, f_trans, f_ip(0), f_ip(1), f_xp(0), f_xp(1),
                    f_delta, f_dx, f_bc]

        def emit_body(i, j, defer):
            """One scan tile; returns the y-matmul closure if defer."""
            cx = ctxs[i]
            deltaA, deltaB = cx["deltaA"], cx["deltaB"]
            dXA, dXB = cx["dXA"], cx["dXB"]
            brep, crep = cx["brep"], cx["crep"]
            jj = j if j < 16 else j - 16
            if j < 8:
                dsl, xsl = deltaA[0:64, :], dXA[0:64, :]
                psel = p0164[0:64, ts(jj % 8, 128)]
            elif j < 16:
                dsl, xsl = deltaA[64:128, :], dXA[64:128, :]
                psel = p0164[64:128, ts(jj % 8, 128)]
            else:
                dsl, xsl = deltaB[:, :], dXB[:, :]
                psel = p0164[0:64, ts(jj % 8, 128)]
            if j == 0:
                cx["ypsA0"] = ppy.tile([128, LC], f32, name="ypsA0",
                                       tag="ypsA0")
                cx["ypsA1"] = ppy.tile([128, LC], f32, name="ypsA1",
                                       tag="ypsA1")
                cx["ypsBp"] = ppy.tile([128, LC], f32, name="ypsBp",
                                       tag="ypsBp")
            dA_t = pT.tile([128, LCF], bf16, name="dA_t")
            dBu = pT.tile([128, LCF], bf16, name="dBu")
            for h in range(2):
                drep = mmtile(128, LC, "drep")
                nc.tensor.matmul(drep[:], psel, dsl[:, bass.ds(h * LC, LC)],
                                 start=True, stop=True)
                nc.scalar.activation(dA_t[:, ts(h, LC)], drep[:], AF.Exp,
                                     scale=acol[:, j:j + 1])
                dxrep = mmtile(128, LC, "dxrep")
                nc.tensor.matmul(dxrep[:], psel, xsl[:, bass.ds(h * LC, LC)],
                                 start=True, stop=True)
                # dBu = dxrep * brep straight from PSUM: skips the scalar
                # staging copy and its SBUF round-trip
                nc.vector.tensor_tensor(dBu[:, ts(h, LC)], dxrep[:],
                                        brep[:, ts(h, LC)], op=OP.mult)
            h_t = pT.tile([128, LCF], bf16, name="h_t")
            init = 0.0 if i == 0 else hlast[:, j:j + 1]
            nc.vector.tensor_tensor_scan(
                h_t[:], dA_t[:], dBu[:], init, op0=OP.mult, op1=OP.add)
            nc.vector.tensor_copy(hlast[:, j:j + 1], h_t[:, LCF - 1:LCF])
            ch = pT.tile([128, LCF], bf16, name="ch")
            nc.vector.tensor_tensor(ch[:], h_t[:], crep[:], op=OP.mult)

            def y_mm():
                if j < 16:
                    nc.tensor.matmul(cx["ypsA0"][:], r01all[:, ts(jj, 128)],
                                     ch[:, 0:LC], start=(j == 0), stop=False)
                    nc.tensor.matmul(cx["ypsA1"][:], r01all[:, ts(jj, 128)],
                                     ch[:, LC:LCF], start=(j == 0), stop=False)
                else:
                    nc.tensor.matmul(cx["ypsBp"][0:DB, :],
                                     r01ball[:, ts(jj, 64)],
                                     ch[:, 0:LC], start=(j == 16), stop=False,
                                     skip_group_check=True)
                    nc.tensor.matmul(cx["ypsBp"][DB:128, :],
                                     r01ball[:, ts(jj, 64)],
                                     ch[:, LC:LCF], start=(j == 16),
                                     stop=False, skip_group_check=True)
            if defer:
                return y_mm
            y_mm()
            return None

        def emit_tail(i):
            cx = ctxs[i]
            ypsA0, ypsA1 = cx["ypsA0"], cx["ypsA1"]
            ypsB0 = cx["ypsBp"][0:DB, :]
            ypsB1 = cx["ypsBp"][DB:128, :]
            # Dp residual folded into the y accumulation as a diagonal matmul
            yA = pT.tile([DA, LCF], bf16, name="yA", bufs=1)
            yB = pT.tile([DB, LCF], bf16, name="yB", bufs=1)
            for h, (ya_ps, yb_ps) in enumerate([(ypsA0[:], ypsB0),
                                                (ypsA1[:], ypsB1)]):
                sl = bass.ds(i * LCF + h * LC, LC)
                nc.tensor.matmul(ya_ps, dpdA[:], xi2A[:, sl],
                                 start=False, stop=True, skip_group_check=True)
                nc.tensor.matmul(yb_ps, dpdB[:], xi2B[:, sl],
                                 start=False, stop=True, skip_group_check=True)
                nc.vector.tensor_tensor(yA[:, ts(h, LC)], ya_ps,
                                        zsA[:, sl], op=OP.mult)
                nc.vector.tensor_tensor(yB[:, ts(h, LC)], yb_ps,
                                        zsB[:, sl], op=OP.mult)
            mout = pT.tile([C, LCF], bf16, name="mout", bufs=1)
            for h in range(2):
                mps = mmtile(C, LC, "mps")
                nc.tensor.matmul(mps[:], outwTA[:], yA[:, ts(h, LC)],
                                 start=True, stop=False)
                nc.tensor.matmul(mps[:], outwTB[:], yB[:, ts(h, LC)],
                                 start=False, stop=True)
                nc.scalar.copy(mout[:, ts(h, LC)], mps[:])
            # Z = xdq + mout^T computed wholly in PSUM
            for r0 in range(0, 16, 4):
                zps = pmm.tile([WQ, 4 * C], f32, name="zps", tag="mm")
                zps3 = zps.rearrange("p (r c) -> p r c", r=4)
                for k in range(4):
                    r = r0 + k
                    nc.tensor.matmul(zps3[:, k, :], mout[:, ts(r, WQ)],
                                     identb[:], start=True, stop=False)
                    nc.tensor.matmul(zps3[:, k, :], identb[0:WQ, 0:WQ],
                                     xdq3[:, :, 16 * i + r],
                                     start=False, stop=True)
                hq0 = 16 * i + r0
                nc.scalar.activation(Z3[:, :, hq0:hq0 + 4],
                                     zps3.transpose([0, 2, 1]), AF.Copy)

        # serial front-ends (concurrent-traffic inflation made interleaved
        # emission a net loss: SBUF ports, not engine cycles, are the wall);
        # only the 3-body tail deferral is kept to cover the PE-heavy tail
        fronts = [make_front(i) for i in range(NI)]
        for f in fronts[0]:
            f()
        start_j = 0
        for i in range(NI):
            for j in range(start_j, NT):
                emit_body(i, j, defer=False)
            if i + 1 < NI:
                for f in fronts[i + 1]:
                    f()
                deferred = [emit_body(i + 1, jd, defer=True) for jd in range(3)]
                start_j = 3
            else:
                deferred = []
            emit_tail(i)
            for fn in deferred:
                fn()
        pT.release()
        pC.release()
        pE.release()
        pG.release()

        # =============== Phase G: IDCT contribution ===============
        pH = tc.alloc_tile_pool(name="pH", bufs=1)
        # t7[hq, c, W] = sum_wq Z[wq, c, hq] * Mw_q[wq, W]  (per-c matmul)
        t7 = pH.tile([HQ, C * W], bf16)
        t7_3 = t7.rearrange("p (c w) -> p c w", c=C)
        for ci, c0 in enumerate(range(0, C, 4)):
            t7ps = tptile(HQ, 4 * W, "t7ps")
            t7ps3 = t7ps.rearrange("p (c w) -> p c w", c=4)
            for k in range(4):
                nc.tensor.matmul(t7ps3[:, k, :], Z3[:, c0 + k, :],
                                 mwq64[64:128, :], start=True, stop=True)
            if ci % 2 == 0:
                nc.scalar.activation(t7_3[:, c0:c0 + 4, :], t7ps3[:, :, :],
                                     AF.Copy)
            else:
                nc.vector.tensor_copy(t7_3[:, c0:c0 + 4, :], t7ps3[:, :, :])
        ctr_h = contrib
        pSo = tc.alloc_tile_pool(name="pSo", bufs=3)
        for i in range(24):
            cps = mmtile(H, LC, "cps")
            nc.tensor.matmul(cps[:], mhq[:], t7[:, ts(i, LC)], start=True, stop=True)
            csb = pSo.tile([H, LC], bf16, name="csb")
            if i % 2 == 0:
                nc.scalar.activation(csb[:], cps[:], AF.Copy)
            else:
                nc.vector.tensor_copy(csb[:], cps[:])
            nc.sync.dma_start(
                ctr_h[:, ts(i, 4), :],
                csb[:].rearrange("h (c w) -> h c w", c=4))
        pSo.release()
        pH.release()
        pD_.release()
        ptp.release()
        ppy.release()
        pmm.release()
        consts.release()

    nc.compile()
    return nc


def _host_inputs(inputs):
    """Build the 8 per-core input maps."""
    x = inputs["x"]
    ln_w, ln_b = inputs["ln_w"], inputs["ln_b"]
    Mh = _dct_mat(H)
    Mw = _dct_mat(W)
    ident = np.eye(128, dtype=np.float32)
    # within-tile partition order p = 8*s + r (r = d offset, s = state):
    # makes row replication p -> p%8 a contiguous-partition doubling
    s01 = np.zeros((S, 128), np.float32)
    p0164 = np.zeros((128, 128 * 8), np.float32)
    r01all = np.zeros((128, 128 * 16), np.float32)
    r01ball = np.zeros((128, 64 * 8), np.float32)
    for p in range(128):
        s01[p // 8, p] = 1.0
        for j in range(8):
            p0164[8 * j + p % 8, 128 * j + p] = 1.0
            p0164[64 + 8 * j + p % 8, 128 * j + p] = 1.0
        for j in range(16):
            r01all[p, 128 * j + 8 * j + p % 8] = 1.0
        for j in range(8):
            r01ball[p, 64 * j + 8 * j + p % 8] = 1.0
    in_maps = []
    for k in range(8):
        b, q = k // 4, k % 4
        h0 = (q // 2) * HQ
        w0 = (q % 2) * WQ
        in_w2 = (inputs["in_w"][q] * ln_w[None, :]).astype(np.float32)
        bias_e = (inputs["in_w"][q] @ ln_b).astype(np.float32)
        # conv folded into in_proj: tap k' applies weight conv_w[:, 3-k'] to
        # positions shifted back by k'
        convw = inputs["conv_w"][q].astype(np.float32)        # [D, 4]
        convb = inputs["conv_b"][q].astype(np.float32)        # [D]
        w_rev = convw[:, ::-1]
        WX = in_w2[:D]                                        # [D, C]
        inwtap = np.concatenate(
            [(WX * w_rev[:, kk][:, None]).T for kk in range(KCONV)], axis=1)
        bconv = bias_e[:D] * convw.sum(1) + convb
        bc3 = np.stack(
            [bias_e[:D] * w_rev[:, :t + 1].sum(1) + convb
             for t in range(KCONV - 1)], axis=1)              # [D, 3]
        xpw80 = np.zeros((D, 80), np.float32)
        xpwT = inputs["xp_w"][q].T
        xpw80[:, 0:RK] = xpwT[:, 0:RK]
        xpw80[:, 32:32 + S] = xpwT[:, RK:RK + S]
        xpw80[:, 64:64 + S] = xpwT[:, RK + S:RK + 2 * S]
        A = (-np.exp(inputs["A_log"][q])).astype(np.float32)  # [D, S]
        acol = np.zeros((128, NT), np.float32)
        for j in range(NT):
            for p in range(128):
                acol[p, j] = A[j * 8 + p % 8, p // 8]
        m = {
            "xbh": np.ascontiguousarray(x[b].transpose(1, 0, 2)),
            "mhqT": Mh[h0:h0 + HQ, :].T,
            "mwqT": Mw[w0:w0 + WQ, :].T,
            "mhq": Mh[h0:h0 + HQ, :],
            "mwq": Mw[w0:w0 + WQ, :],
            "ident": ident,
            "inwtap": inwtap,
            "inwz": in_w2[D:].T,
            "biasz": bias_e[D:, None],
            "bconv": bconv[:, None],
            "bc3": bc3,
            "xpwT": xpw80,
            "dtwT": inputs["dt_w"][q].T,
            "dtb": inputs["dt_b"][q][:, None],
            "acol": acol,
            "dpdA": np.diag(inputs["Dp"][q][:DA]),
            "dpdB": np.diag(inputs["Dp"][q][DA:]),
            "outwT": inputs["out_w"][q].T,
            "p0164": p0164,
            "s01": s01,
            "r01all": r01all,
            "r01ball": r01ball,
        }
        import ml_dtypes
        bf = ["inwtap", "inwz", "xpwT", "dtwT", "outwT", "p0164", "s01",
              "dpdA", "dpdB",
              "r01all", "r01ball", "xbh", "mhqT", "mwqT", "mhq", "mwq",
              "contrib"]
        in_maps.append({
            kk: np.ascontiguousarray(np.asarray(
                vv, ml_dtypes.bfloat16 if kk in bf else np.float32))
            for kk, vv in m.items()})
    return in_maps


def kernel(**inputs):
    from concourse import bass_utils
    inputs = {k: np.asarray(v) for k, v in inputs.items()}
    if "nc" not in _BUILT:
        _BUILT["nc"] = _build_nc()
    nc = _BUILT["nc"]
    in_maps = _host_inputs(inputs)
    res = bass_utils.run_bass_kernel_spmd(nc, in_maps, core_ids=list(range(8)))
    out = np.zeros((B, C, H, W), np.float32)
    for k in range(8):
        out[k // 4] += np.asarray(
            res.results[k]["contrib"], np.float32).transpose(1, 0, 2)
    return out


if __name__ == "__main__":
    # smoke: random inputs, shape check only
    rng = np.random.default_rng(0)
    demo = {
        "x": rng.standard_normal((B, C, H, W), np.float32),
        "ln_w": np.ones(C, np.float32), "ln_b": np.zeros(C, np.float32),
        "in_w": rng.standard_normal((4, 2 * D, C), np.float32) * 0.02,
        "conv_w": rng.standard_normal((4, D, KCONV), np.float32) * 0.02,
        "conv_b": np.zeros((4, D), np.float32),
        "xp_w": rng.standard_normal((4, RK + 2 * S, D), np.float32) * 0.02,
        "dt_w": rng.standard_normal((4, D, RK), np.float32) * 0.02,
        "dt_b": np.full((4, D), -4.0, np.float32),
        "A_log": np.tile(np.log(np.arange(1, S + 1, dtype=np.float32)), (4, D, 1)),
        "Dp": np.ones((4, D), np.float32),
        "out_w": rng.standard_normal((4, C, D), np.float32) * 0.02,
    }
    out = kernel(**demo)
    print("kernel output:", out.shape, out.dtype)

